# revision 26
# baseline (speedup 1.0000x reference)
"""Causal self-attention (B=4, T=2048, C=768, 12 heads) on 8 Trainium2 cores.

Sharding: core i handles batch b = i//2 and head-set s = i%2 (6 of 12 heads).
Each core computes x[b] @ W_attn slice -> 6 heads of causal attention -> a
partial projection (row-sharded W_proj).  The host sums the two partials per
batch and adds b_proj.  Measured 201.1us on HW (v2 baseline: 267.7us).

Design notes (trace-driven; see per-block comments):
  - All matmul operands bf16.  Q^T/K^T in pair layout [128, T] (head a on
    partitions 0-63, b on 64-127); the two S^T matmuls of a pair run
    concurrently on the row-split PE array.  1/sqrt(64) folded into W_q.
  - V' [T, 6*65] with an all-ones column per head: the PV matmul yields
    Y'^T and the softmax denominator row together.
  - exp on ScalarE out of PSUM in [128, 1024] pair tiles with exact causal
    shrink; the causal mask is a single [128, 256] unit triangle (the
    consumed sub-block is identical for every diagonal position).
  - Normalize without PE transposes or row-form reciprocals (both measured
    slow: 192 transposes cost ~51us of PE; DVE recip is ~6.4ns/elem per
    LANE, so [1,512] rows cost 3.3us each): a tiny SBUF->SBUF DMA spreads
    the denominator row across 8 partitions, DVE reciprocal at free-size
    128, DMA scatter back, GpSimd partition_broadcast, DVE multiply.
  - kc-granular fill scheduling: Q/K projections, V' tiles and output-
    projection halves are emitted inside the attention kc-loops right
    after each exp, so the PE executes them while ScalarE streams the
    1.1us activation.  Unit-granularity interleaving leaves the PE with
    cold-clock deserts (HAM re-throttles to 1.2GHz) and starves ScalarE
    at chunk boundaries.
  - Inputs land via ~15 multi-dim-AP DMAs split across the Sync and
    Scalar HWDGE queues (issue costs ~650ns of queue time each; per-queue
    streaming is only ~70-120GB/s, so the first unit's tiles go first and
    x is pre-blocked on the host for contiguous rows).
  - HAM warmup matmuls on the first-landed tile at the head, and
    data-anchored bf16 keepwarms across the last normalize chain (the
    scheduler hoists dependency-free warmers away from the tail).
  - Output staged and stored as bf16 per projection half (host
    accumulates the two partials per batch in f32).
"""

import numpy as np

import concourse.bass as bass
import concourse.mybir as mybir
import concourse.tile as tile
from concourse import bacc

B, T, C = 4, 2048, 768
NH, HD = 12, 64
N_CORES = 8
HPC = 6  # heads per core
P = 128
F32 = mybir.dt.float32
BF16 = mybir.dt.bfloat16
QC_N = T // 512  # 4 q-chunks of 512
KC_N = T // P    # 16 k-chunks of 128
CKC = C // P     # 6 contraction chunks for the QKV projection
N_WARM = 10      # HAM warmup matmuls


def build_program(n_iters: int = 1):
    """Builds the SPMD program (identical on all cores; data differs)."""
    nc = bacc.Bacc(
        "TRN2",
        target_bir_lowering=False,
        debug=False,
        enable_asserts=False,
        num_devices=N_CORES,
    )
    d_xt = nc.dram_tensor("xt", [QC_N, C, 512], BF16, kind="ExternalInput").ap()
    d_wq = nc.dram_tensor("wq", [3, C, P], BF16, kind="ExternalInput").ap()
    d_wk = nc.dram_tensor("wk", [3, C, P], BF16, kind="ExternalInput").ap()
    d_wv = nc.dram_tensor("wv", [C, 390], BF16, kind="ExternalInput").ap()
    d_w2 = nc.dram_tensor("w2", [384, C], BF16, kind="ExternalInput").ap()
    d_masks = nc.dram_tensor("masks", [P, 256], BF16, kind="ExternalInput").ap()
    d_out = nc.dram_tensor("out", [T, C], BF16, kind="ExternalOutput").ap()

    with tile.TileContext(nc) as tc:
        const_cm = tc.tile_pool(name="const", bufs=1)
        work_cm = tc.tile_pool(name="work", bufs=1)
        sb_cm = tc.tile_pool(name="sbw", bufs=2)
        ps_cm = tc.tile_pool(name="psum", bufs=1, space="PSUM")
        const = const_cm.__enter__()
        work = work_cm.__enter__()
        sbw = sb_cm.__enter__()
        psp = ps_cm.__enter__()

        def body(_i=None):
            # ---- persistent tiles ----
            wq_sb = const.tile([P, CKC, 384], BF16, tag="wq")
            wk_sb = const.tile([P, CKC, 384], BF16, tag="wk")
            wv_sb = const.tile([P, CKC, 390], BF16, tag="wv")
            w2_sb = const.tile([P, 3, C], BF16, tag="w2")
            masks_sb = const.tile([P, 256], BF16, tag="masks")
            xt_sb = work.tile([P, CKC, T], BF16, tag="xt")
            qt_sb = [work.tile([P, T], BF16, tag=f"qt{p}", name=f"qtp{p}") for p in range(3)]
            kt_sb = [work.tile([P, T], BF16, tag=f"kt{p}", name=f"ktp{p}") for p in range(3)]
            v_sb = [work.tile([P, 390], BF16, tag=f"v{t}", name=f"v{t}") for t in range(KC_N)]
            yn_sb = [work.tile([P, T], BF16, tag=f"yn{p}", name=f"yn{p}") for p in range(3)]

            # ---- combined loads, in first-use order, on two HWDGE queues.
            # Per-queue streaming runs at only ~70-120 GB/s, so the loads
            # that gate the first attention unit (xt block 0, pair-0 Q/K
            # weight slices, V weights, the 64KB triangle mask) are split
            # finely across both queues and everything else follows.
            d_wv3 = d_wv.rearrange("(k p) n -> p k n", p=P)

            def xt_block(b, klo, khi):
                return (xt_sb[:, klo:khi, b * 512:(b + 1) * 512],
                        d_xt[b, klo * P:khi * P, :].rearrange(
                            "(k p) t -> p k t", p=P))

            def w_slice(d_w, w_sb, pp):
                return (w_sb[:, :, pp * P:(pp + 1) * P],
                        d_w[pp].rearrange("(k p) c -> p k c", p=P))

            nc.sync.dma_start(*xt_block(0, 0, 3))
            nc.sync.dma_start(*w_slice(d_wk, wk_sb, 0))
            nc.sync.dma_start(wv_sb[:, 0, :], d_wv3[:, 0, :])
            nc.sync.dma_start(wv_sb[:, 1:, :], d_wv3[:, 1:, :])
            nc.sync.dma_start(*w_slice(d_wk, wk_sb, 1))
            nc.sync.dma_start(*w_slice(d_wk, wk_sb, 2))
            for qq in range(1, QC_N):
                nc.sync.dma_start(*xt_block(qq, 0, CKC))
            nc.scalar.dma_start(masks_sb[:], d_masks[:])
            nc.scalar.dma_start(*w_slice(d_wq, wq_sb, 0))
            nc.scalar.dma_start(*xt_block(0, 3, 6))
            nc.scalar.dma_start(*w_slice(d_wq, wq_sb, 1))
            nc.scalar.dma_start(*w_slice(d_wq, wq_sb, 2))
            nc.scalar.dma_start(w2_sb[:], d_w2.rearrange("(k p) n -> p k n", p=P))

            # ---- ACT exp-table preload: a dummy exp as soon as the first
            # weight tile lands, so the first real exp skips the ~2.7us
            # ACT_TABLE_LOAD.
            dumm = sbw.tile([1, 2], BF16, tag="dumm", bufs=1)
            nc.scalar.activation(
                dumm[:], masks_sb[0:1, 0:2], mybir.ActivationFunctionType.Exp
            )

            # ---- HAM warmup: keep the PE busy while the real inputs land.
            # Results are never read; wv values (~0.02 scale) stay finite.
            warm_ps = psp.tile([P, 256], F32, tag="misc", bufs=2, name="warm")
            for _ in range(N_WARM):
                nc.tensor.matmul(
                    warm_ps[:],
                    lhsT=masks_sb[:, 0:P],
                    rhs=masks_sb[:],
                    start=True,
                    stop=True,
                )

            def v_tile(t):
                """V' tile for k-chunk t (+ ones column per head)."""
                ps = psp.tile([P, 512], F32, tag="misc", bufs=2,
                              name=f"vps{t}")
                for k in range(CKC):
                    nc.tensor.matmul(
                        ps[:, :390],
                        lhsT=xt_sb[:, k, t * P:(t + 1) * P],
                        rhs=wv_sb[:, k, :],
                        start=(k == 0),
                        stop=(k == CKC - 1),
                    )
                nc.vector.tensor_copy(v_sb[t][:], ps[:, :390])
                # ones columns on DVE: GpSimd memset would thrash the Q7
                # ucode library against partition_broadcast
                nc.vector.memset(
                    v_sb[t].rearrange("p (h c) -> p h c", h=HPC)[:, :, HD:],
                    1.0,
                )

            def qk_half(p, qc, which):
                """One of the Q^T / K^T projections for pair p, q-chunk qc."""
                w_sb, o_sb = ((wq_sb, qt_sb), (wk_sb, kt_sb))[which]
                ps = psp.tile([P, 512], F32, tag="misc", bufs=2,
                              name=f"qk{qc}{p}{which}")
                for k in range(CKC):
                    nc.tensor.matmul(
                        ps[:],
                        lhsT=w_sb[:, k, p * P:(p + 1) * P],
                        rhs=xt_sb[:, k, qc * 512:(qc + 1) * 512],
                        start=(k == 0),
                        stop=(k == CKC - 1),
                    )
                nc.vector.tensor_copy(
                    o_sb[p][:, qc * 512:(qc + 1) * 512], ps[:]
                )

            proj_obs = {}

            def proj_half(qb, which):
                """Half of the output projection for row-block qb."""
                if which == 0:
                    ob = sbw.tile([P, C], BF16, tag="ob", bufs=4)
                    proj_obs[qb] = ob
                    n0, nw = 0, 512
                else:
                    ob = proj_obs.pop(qb)
                    n0, nw = 512, 256
                po = psp.tile([P, 512], F32, tag="misc", bufs=2,
                              name=f"po{qb}{which}")
                for pp in range(3):
                    nc.tensor.matmul(
                        po[:, :nw],
                        lhsT=yn_sb[pp][:, qb * P:(qb + 1) * P],
                        rhs=w2_sb[:, pp, n0:n0 + nw],
                        start=(pp == 0),
                        stop=(pp == 2),
                    )
                nc.vector.tensor_copy(ob[:, n0:n0 + nw], po[:, :nw])
                nc.sync.dma_start(
                    d_out[qb * P:(qb + 1) * P, n0:n0 + nw], ob[:, n0:n0 + nw]
                )

            def attn_pair(p, qc, fills=()):
                    # ---- causal attention for (pair p, q-chunk qc) ----
                    # fills: per-kc lists of pure-PE work emitted right
                    # after each exp, so it executes while ScalarE streams
                    # the 1.1us activation instead of piling up at unit
                    # boundaries (where it runs cold and starves ScalarE).
                    n_kc = 4 * qc + 4
                    yps = [psp.tile([P, 512], F32, tag="yp", bufs=2,
                                    name=f"yp{qc}{p}{h2}") for h2 in range(2)]
                    for kc in range(n_kc):
                        m = kc - 4 * qc
                        s0 = 128 * max(m, 0)   # first live q-col in chunk
                        ss = psp.tile([P, 1024], F32, tag="ss", bufs=2)
                        for h2 in range(2):
                            pb = 64 * h2
                            nc.tensor.matmul(
                                ss[:, h2 * 512 + s0:(h2 + 1) * 512],
                                lhsT=kt_sb[p][pb:pb + 64, kc * P:(kc + 1) * P],
                                rhs=qt_sb[p][pb:pb + 64,
                                             qc * 512 + s0:(qc + 1) * 512],
                                start=True,
                                stop=True,
                            )
                        pt = sbw.tile([P, 1024], BF16, tag="pt", bufs=4)
                        if s0:
                            ss_r = ss.rearrange("p (h c) -> p h c", h=2)
                            pt_r = pt.rearrange("p (h c) -> p h c", h=2)
                            nc.scalar.activation(
                                pt_r[:, :, s0:], ss_r[:, :, s0:],
                                mybir.ActivationFunctionType.Exp,
                            )
                        else:
                            nc.scalar.activation(
                                pt[:], ss[:], mybir.ActivationFunctionType.Exp
                            )
                        if kc < len(fills):
                            for f in fills[kc]:
                                f()
                        if m >= 0:
                            # masked multiply: only cols [s0, s0+128) can
                            # violate causality (beyond that all 128 k-rows
                            # are below the diagonal)
                            pt_r = pt.rearrange("p (h c) -> p h c", h=2)
                            mk_r = masks_sb.rearrange("p (h c) -> p h c", h=2)
                            nc.vector.tensor_tensor(
                                pt_r[:, :, s0:s0 + 128],
                                pt_r[:, :, s0:s0 + 128],
                                mk_r[:],
                                mybir.AluOpType.mult,
                            )
                        for h2 in range(2):
                            ch = p * 2 + h2
                            nc.tensor.matmul(
                                yps[h2][:65, s0:],
                                lhsT=v_sb[kc][:, ch * 65:(ch + 1) * 65],
                                rhs=pt[:, h2 * 512 + s0:(h2 + 1) * 512],
                                start=(kc == 0),
                                stop=(kc == n_kc - 1),
                            )
                        if kc == n_kc - 4:
                            # fully-written pt whose pool slot is never
                            # recycled afterwards: safe keepwarm anchor
                            pt_last = pt
                    # ---- normalize: yn = y * (1/d)  (d = row 64) ----
                    # The [1, 1024] denominator row is spread over 8
                    # partitions by a tiny SBUF->SBUF DMA (8x512B
                    # descriptors), reciprocal'd on DVE at free-size 128
                    # (DVE recip costs ~6.4ns/elem *per partition lane*, so
                    # the row form would cost 3.3us), scattered back, then
                    # partition-broadcast on GpSimd and multiplied.
                    ysb = sbw.tile([65, 1024], F32, tag="ysb", bufs=3)
                    # d-row first: the gather DMA only depends on these two
                    # tiny copies, not the full Y' evacuation.  For the last
                    # unit they go on ScalarE (idle by then, DVE is not) to
                    # shave the tail chain.
                    last = (qc == QC_N - 1 and p == 2)
                    for h2 in range(2):
                        if last:
                            nc.scalar.copy(
                                ysb[64:65, h2 * 512:(h2 + 1) * 512],
                                yps[h2][64:65, :],
                            )
                        else:
                            nc.vector.tensor_copy(
                                ysb[64:65, h2 * 512:(h2 + 1) * 512],
                                yps[h2][64:65, :],
                            )
                    dT2 = sbw.tile([8, P], F32, tag="dT2", bufs=3)
                    nc.sync.dma_start(dT2[:], ysb[64:65, :])
                    for h2 in range(2):
                        nc.vector.tensor_copy(
                            ysb[:64, h2 * 512:(h2 + 1) * 512], yps[h2][:64, :]
                        )
                    rT2 = sbw.tile([8, P], F32, tag="rT2", bufs=3)
                    with nc.allow_low_precision("softmax denom recip"):
                        nc.vector.reciprocal(rT2[:], dT2[:])
                    rrow = sbw.tile([1, 1024], F32, tag="rrow", bufs=3)
                    nc.sync.dma_start(rrow[:], rT2[:])
                    for h2 in range(2):
                        pb = 64 * h2
                        dbc = sbw.tile([64, 512], F32, tag="dbc", bufs=3)
                        nc.gpsimd.partition_broadcast(
                            dbc[:], rrow[0:1, h2 * 512:(h2 + 1) * 512]
                        )
                        if last:
                            # split so the final projection starts per-block
                            for qb in range(4):
                                nc.vector.tensor_tensor(
                                    yn_sb[p][pb:pb + 64,
                                             qc * 512 + qb * 128:qc * 512 + (qb + 1) * 128],
                                    ysb[:64, h2 * 512 + qb * 128:h2 * 512 + (qb + 1) * 128],
                                    dbc[:, qb * 128:(qb + 1) * 128],
                                    mybir.AluOpType.mult,
                                )
                        else:
                            nc.vector.tensor_tensor(
                                yn_sb[p][pb:pb + 64, qc * 512:(qc + 1) * 512],
                                ysb[:64, h2 * 512:(h2 + 1) * 512],
                                dbc[:],
                                mybir.AluOpType.mult,
                            )
                    return ysb, pt_last

            # ---- fill schedule: one thunk-list per kc slot per unit.
            # qk(p,qc) is always emitted one unit ahead of attn(p,qc);
            # proj(qc-1) rides inside qc's units; V' tiles stay ahead of
            # their first consumer.
            def vt(t):
                return lambda: v_tile(t)

            def qkh(p, qc, w):
                return lambda: qk_half(p, qc, w)

            def ph(qb, w):
                return lambda: proj_half(qb, w)

            qk_half(0, 0, 0)
            qk_half(0, 0, 1)
            attn_pair(0, 0, [[vt(0)], [vt(1), qkh(1, 0, 0)],
                             [vt(2), qkh(1, 0, 1)], [vt(3)]])
            attn_pair(1, 0, [[qkh(2, 0, 0)], [qkh(2, 0, 1)]])
            attn_pair(2, 0, [[qkh(0, 1, 0)], [qkh(0, 1, 1)]])
            attn_pair(0, 1, [[qkh(1, 1, 0)], [qkh(1, 1, 1)], [vt(4)],
                             [vt(5)], [vt(6)], [vt(7)], [ph(0, 0)],
                             [ph(0, 1)]])
            attn_pair(1, 1, [[vt(8)], [vt(9)], [qkh(2, 1, 0)],
                             [qkh(2, 1, 1)], [ph(1, 0)], [ph(1, 1)],
                             [ph(2, 0)], [ph(2, 1)]])
            attn_pair(2, 1, [[ph(3, 0)], [ph(3, 1)], [vt(10)], [vt(11)],
                             [qkh(0, 2, 0)], [qkh(0, 2, 1)]])
            attn_pair(0, 2, [[qkh(1, 2, 0)], [qkh(1, 2, 1)], [vt(12)],
                             [vt(13)], [ph(4, 0)], [ph(4, 1)], [ph(5, 0)],
                             [ph(5, 1)], [ph(6, 0)], [ph(6, 1)], [ph(7, 0)],
                             [ph(7, 1)]])
            attn_pair(1, 2, [[qkh(2, 2, 0)], [qkh(2, 2, 1)], [vt(14)],
                             [vt(15)], [qkh(0, 3, 0)], [qkh(0, 3, 1)]])
            # (pair 0, qc 3) hoisted ahead of (pair 2, qc 2): the three
            # 16-kc qc=3 units bunched at the end leave ScalarE 96%-saturated
            # for 51us while qc=2 runs ACT duty 76%; interleaving rebalances
            # exp work into the PE-bound stretch
            attn_pair(0, 3, [[qkh(1, 3, 0)], [qkh(1, 3, 1)]])
            attn_pair(2, 2, [])
            attn_pair(1, 3, [[qkh(2, 3, 0)], [qkh(2, 3, 1)], [ph(8, 0)],
                             [ph(8, 1)], [ph(9, 0)], [ph(9, 1)]])
            ysb_last, pt_last = attn_pair(2, 3, [[ph(10, 0)], [ph(10, 1)],
                                                 [ph(11, 0)], [ph(11, 1)]])
            # keep the PE clock warm across the last normalize chain;
            # anchored on the last unit's ysb so the scheduler cannot hoist
            # them earlier, and on the ss tag so they do not hold the misc
            # slots the projection needs.
            warm2 = psp.tile([P, 1024], F32, tag="ss", bufs=2, name="warm2")
            for _ in range(40):
                nc.tensor.matmul(
                    warm2[:, 0:512],
                    lhsT=pt_last[:, 0:P],
                    rhs=pt_last[:, 0:512],
                    start=True,
                    stop=True,
                )
            for qb in range(12, 16):
                proj_half(qb, 0)
                proj_half(qb, 1)

        if n_iters == 1:
            body()
        else:
            with tc.For_i(0, n_iters, 1) as _i:
                body(_i)

        for cm in (ps_cm, sb_cm, work_cm, const_cm):
            cm.__exit__(None, None, None)

    nc.compile()
    return nc


def shard_inputs(x, W_attn, b_attn, W_proj, b_proj):
    """Builds the 8 per-core input maps (all host-side numpy prep)."""
    import ml_dtypes

    x = np.asarray(x, dtype=np.float32)
    W_attn = np.asarray(W_attn, dtype=np.float32)
    b_attn = np.asarray(b_attn, dtype=np.float32)
    W_proj = np.asarray(W_proj, dtype=np.float32)
    assert not np.any(b_attn), "kernel assumes zero attention bias"
    scale = float(HD) ** -0.5
    bf16 = ml_dtypes.bfloat16

    kl = np.arange(P)[:, None]
    ql = np.arange(P)[None, :]
    tri = (kl <= ql).astype(np.float32)
    masks = np.concatenate([tri, tri], axis=1).astype(bf16)  # [128, 256]

    in_maps = []
    for core in range(N_CORES):
        b = core // 2
        s = core % 2
        heads = [s * HPC + j for j in range(HPC)]
        xt_f = x[b].T  # [C, T]
        xt = np.ascontiguousarray(
            np.stack([xt_f[:, qq * 512:(qq + 1) * 512] for qq in range(4)])
        ).astype(bf16)  # [4, C, 512] blocked

        wq = np.empty((3, C, P), np.float32)
        wk = np.empty((3, C, P), np.float32)
        for p in range(3):
            for h2 in range(2):
                hh = heads[p * 2 + h2]
                dst = slice(h2 * HD, (h2 + 1) * HD)
                wq[p][:, dst] = W_attn[:, hh * HD:(hh + 1) * HD] * scale
                wk[p][:, dst] = W_attn[:, C + hh * HD:C + (hh + 1) * HD]

        wv = np.zeros((C, 390), np.float32)
        for ch in range(HPC):
            hh = heads[ch]
            wv[:, ch * 65:ch * 65 + HD] = (
                W_attn[:, 2 * C + hh * HD:2 * C + (hh + 1) * HD]
            )

        w2 = np.empty((384, C), np.float32)
        for p in range(3):
            for h2 in range(2):
                hh = heads[p * 2 + h2]
                w2[p * P + h2 * HD:p * P + (h2 + 1) * HD, :] = (
                    W_proj[hh * HD:(hh + 1) * HD, :]
                )

        in_maps.append({
            "xt": xt,
            "wq": wq.astype(bf16), "wk": wk.astype(bf16),
            "wv": wv.astype(bf16), "w2": w2.astype(bf16),
            "masks": masks,
        })
    return in_maps


def unshard_outputs(results, b_proj):
    b_proj = np.asarray(b_proj, dtype=np.float32)
    out = np.empty((B, T, C), np.float32)
    for b in range(B):
        out[b] = (
            np.asarray(results[2 * b]["out"], dtype=np.float32)
            + np.asarray(results[2 * b + 1]["out"], dtype=np.float32)
            + b_proj
        )
    return out


_CACHED_NC = None


def kernel(x, W_attn, b_attn, W_proj, b_proj):
    global _CACHED_NC
    from concourse import bass_utils

    if _CACHED_NC is None:
        _CACHED_NC = build_program(1)
    in_maps = shard_inputs(x, W_attn, b_attn, W_proj, b_proj)
    res = bass_utils.run_bass_kernel_spmd(
        _CACHED_NC, in_maps, core_ids=list(range(N_CORES))
    )
    return unshard_outputs(res.results, b_proj)
